# revision 15
# baseline (speedup 1.0000x reference)
"""GalaxyTileDecoder on 8 Trainium2 NeuronCores.

The reference pipeline (linear decode -> zero-pad -> gate -> bilinear
grid_sample -> sum over M=2 sources) collapses algebraically: the sample
grid is a pure per-source translation, sampling the padded 53x53 image at
(y, x) = (i + 2.5 - 4*locs[...,0], j + 2.5 - 4*locs[...,1]).  Folding the
integer shift (one-hot over 6 positions per axis), the bilinear weights,
the decoder bias, the galaxy_bool gate, and the M-source sum into an
expanded feature dimension turns the whole forward into one matmul:

    out[p, :] = (sum_src z_exp[p, src, :]) @ W_exp        (K=324)

with W_exp[(a, b, f), (i, j)] = canvas9[f, a+i, b+j] the 6x6 shifted
52x52 windows of the 9 basis images (8 decoder rows + bias) in a 57x57
zero canvas, and z_exp the per-source sparse coefficients
bool * z9[f] * wy[a] * wx[b].

K is split into 3 chunks of 108 rows (a in {0,1} / {2,3} / {4,5}).  A
ptile's z_exp only touches the chunks its live sources' integer y-shifts
hit (dead sources -- galaxy_bool=0 -- touch none), so ptiles are sorted
by chunk-set class and dealt round-robin to the 8 cores; each 128-row
batch then only runs matmuls for the union of chunk-sets present, a
data-dependent schedule baked into the compiled program (rebuilt if the
input distribution ever changes).  The class order makes batch 0 a
single-chunk batch so the PE starts after a minimal input prefix, and
all-dead batches are skipped entirely (host writes those rows as zero).
z is stored chunk-major and densely packed (only batches that use a
chunk get columns).  Output is written bf16 (one 5.4KB DMA descriptor
per partition row) and upconverted on host.

Data parallel over the ptile axis: 1250 ptiles per core, no collectives.
"""

import math
import os

import numpy as np

P_TOTAL = 10000
M = 2
N_CORES = 8
PT = P_TOTAL // N_CORES          # ptiles per core
F = 9                            # 8 decoder features + bias
A = 6                            # y-shift positions
B = 6                            # x-shift positions
K = A * B * F                    # 324 expanded features
CHUNK = 2 * B * F                # 108 rows per K-chunk (2 a-values)
NCH = 3
OUT_HW = 52
COLS = OUT_HW * OUT_HW           # 2704
CANVAS = 57
NB = math.ceil(PT / 128)         # 10 batches per core

_DT_NAME = os.environ.get("BASS_GAL_DT", "bf16")

# class order: a large single-chunk class first (early PE start on a
# minimal input prefix), adjacent classes share chunks, dead last
_CLASS_ORDER = [frozenset(s) for s in
                [{1}, {1, 2}, {2}, {0, 1, 2}, {0, 2}, {0, 1}, {0}, set()]]
_CLASS_RANK = {cs: i for i, cs in enumerate(_CLASS_ORDER)}

_cache = {}


def _build_program(dt_name, sched):
    import concourse.bass as bass  # noqa: F401  (registers engines)
    import concourse.tile as tile
    from concourse import bacc, mybir

    DT = {"bf16": mybir.dt.bfloat16, "f32": mybir.dt.float32}[dt_name]
    WARMUP = int(os.environ.get("BASS_GAL_WARMUP", "40"))

    # chunk -> list of batches using it; z cols for (bi, ci) start at
    # 128 * index_of(bi in zbatches[ci])
    zbatches = {c: [bi for bi in range(NB) if c in sched[bi]]
                for c in range(NCH)}
    zoff = {c: sum(len(zbatches[cc]) for cc in range(c)) * 128
            for c in range(NCH)}
    ZW = max(1, sum(len(v) for v in zbatches.values())) * 128

    nc = bacc.Bacc(trn_type="TRN2")
    zt = nc.dram_tensor("zt", [CHUNK, ZW], DT, kind="ExternalInput")
    wx = nc.dram_tensor("wx", [CHUNK, NCH * COLS], DT, kind="ExternalInput")
    out = nc.dram_tensor("out", [NB * 128, COLS], DT, kind="ExternalOutput")

    PIECES = [(0, 1024), (1024, 2048), (2048, COLS)]

    used = sorted({c for S in sched for c in S})
    # first-use order of chunks
    load_order = []
    for S in sched:
        for c in S:
            if c not in load_order:
                load_order.append(c)
    for c in used:
        if c not in load_order:
            load_order.append(c)
    live = [bi for bi in range(NB) if sched[bi]]
    last_live = live[-1] if live else -1
    H = CHUNK // 2

    with tile.TileContext(nc) as tc:
        with (
            tc.tile_pool(name="w", bufs=1) as wpool,
            tc.tile_pool(name="o", bufs=3) as opool,
            tc.tile_pool(name="ps", bufs=4, space="PSUM") as pspool,
        ):
            # PE warmup: dummy matmuls spanning the input-load phase so the
            # HAM clock-gate is at 2.4 GHz when the real matmuls start.
            warm = wpool.tile([128, 128], mybir.dt.bfloat16, tag="warm")
            nc.vector.memset(warm[:], 0.0)
            wps = pspool.tile([128, 1024], mybir.dt.float32, tag="ps")
            for _ in range(WARMUP):
                nc.tensor.matmul(wps[:, 0:64], warm[:, 0:128], warm[:, 0:64],
                                 start=True, stop=True)
            # inputs in first-use order.  One DMA instruction only engages
            # ~3 of the 16 DMA rings (~87 GB/s), so each w tile is split
            # into 4 instructions (partition halves x column halves) over
            # both HWDGE queues; z left halves (needed by early batches)
            # load up front, z right halves at the end.
            z_sb, w_sb, z_tail = {}, {}, []
            CH2 = COLS // 2
            for c in load_order:
                zw = len(zbatches[c]) * 128
                zh = min(zw, max(128, (zw // 2) // 128 * 128))
                zb = wpool.tile([CHUNK, zw], DT, tag=f"z{c}")
                nc.sync.dma_start(zb[:, 0:zh], zt[:, zoff[c]:zoff[c] + zh])
                if zh < zw:
                    z_tail.append((zb, zh, zw, c))
                z_sb[c] = zb
                wt = wpool.tile([CHUNK, COLS], DT, tag=f"w{c}")
                c0 = c * COLS
                nc.sync.dma_start(wt[0:H, 0:CH2], wx[0:H, c0:c0 + CH2])
                nc.scalar.dma_start(wt[H:CHUNK, 0:CH2],
                                    wx[H:CHUNK, c0:c0 + CH2])
                nc.sync.dma_start(wt[0:H, CH2:COLS],
                                  wx[0:H, c0 + CH2:c0 + COLS])
                nc.scalar.dma_start(wt[H:CHUNK, CH2:COLS],
                                    wx[H:CHUNK, c0 + CH2:c0 + COLS])
                w_sb[c] = wt
            for zb, zh, zw, c in z_tail:
                nc.scalar.dma_start(zb[:, zh:zw],
                                    zt[:, zoff[c] + zh:zoff[c] + zw])
            # preload the Activation table used by scalar copies (after the
            # input DMA issues -- the load occupies the sequencer ~1.3us)
            nc.scalar.copy(warm[0:1, 0:1], warm[0:1, 2:3])

            ei = 0
            for bi in range(NB):
                S = sched[bi]
                if not S:
                    continue  # all-dead batch: host fills zeros
                b0 = bi * 128
                ordS = [c for c in load_order if c in S]
                osb = opool.tile([128, COLS], DT, tag="osb")
                for pi, (p0, p1) in enumerate(PIECES):
                    pw = p1 - p0
                    ps = pspool.tile([128, 1024], mybir.dt.float32, tag="ps")
                    for idx, ci in enumerate(ordS):
                        zcol = zoff_local = zbatches[ci].index(bi) * 128
                        # moving width <=512: PSUM accumulation regions are
                        # limited to one 2KB bank
                        for s0 in range(p0, p1, 512):
                            s1 = min(s0 + 512, p1)
                            nc.tensor.matmul(
                                ps[:, s0 - p0:s1 - p0],
                                z_sb[ci][:, zcol:zcol + 128],
                                w_sb[ci][:, s0:s1],
                                start=(idx == 0),
                                stop=(idx == len(ordS) - 1),
                            )
                    # alternate psum->sbuf (+ bf16 downconvert) between the
                    # DVE and Activation engines so copies hide under matmuls
                    if bi == last_live and pi == 2:
                        # final piece: split copy across both engines and
                        # drain on both HWDGE queues to minimize the tail
                        mid = p0 + 512
                        nc.vector.tensor_copy(osb[:, p0:mid], ps[:, 0:512])
                        nc.scalar.copy(osb[:, mid:p1], ps[:, 512:pw])
                        nc.sync.dma_start(out[b0:b0 + 128, p0:mid],
                                          osb[:, p0:mid])
                        nc.scalar.dma_start(out[b0:b0 + 128, mid:p1],
                                            osb[:, mid:p1])
                        ei += 1
                        continue
                    if ei % 2 == 0:
                        nc.vector.tensor_copy(osb[:, p0:p1], ps[:, 0:pw])
                    else:
                        nc.scalar.copy(osb[:, p0:p1], ps[:, 0:pw])
                    ei += 1
                    if bi == last_live:
                        # drain the final batch piece by piece to shorten
                        # the tail after the last matmul; all stores go on
                        # the SP queue (the Activation sequencer is kept
                        # free for copies)
                        nc.sync.dma_start(out[b0:b0 + 128, p0:p1],
                                          osb[:, p0:p1])
                if bi != last_live:
                    nc.sync.dma_start(out[b0:b0 + 64, :], osb[0:64, :])
                    nc.sync.dma_start(out[b0 + 64:b0 + 128, :],
                                      osb[64:128, :])
    nc.compile()
    return nc


def _get_program(dt_name, sched):
    key = (dt_name, sched)
    if key not in _cache:
        _cache[key] = _build_program(dt_name, sched)
    return _cache[key]


def _host_expand(locs, galaxy_params, galaxy_bool, W_dec, b_dec, np_dtype):
    """Expanded coefficients (sorted/dealt/dense-packed), W_exp, schedule."""
    locs = np.asarray(locs, np.float32).reshape(P_TOTAL, M, 2)
    params = np.asarray(galaxy_params, np.float32).reshape(P_TOTAL * M, 8)
    gbool = np.asarray(galaxy_bool, np.float32).reshape(P_TOTAL * M, 1)
    W = np.asarray(W_dec, np.float32)
    b = np.asarray(b_dec, np.float32)
    N = P_TOTAL * M

    sy = 2.5 - 4.0 * locs[..., 0].reshape(N)
    sx = 2.5 - 4.0 * locs[..., 1].reshape(N)
    m = np.floor(sy)
    k = np.floor(sx)
    fy = (sy - m).astype(np.float32)
    fx = (sx - k).astype(np.float32)
    m = m.astype(np.int64)
    k = k.astype(np.int64)
    assert m.min() >= -2 and m.max() <= 2 and k.min() >= -2 and k.max() <= 2
    ar = np.arange(N)
    cy = np.zeros((N, A), np.float32)
    cx = np.zeros((N, B), np.float32)
    cy[ar, m + 2] = 1.0 - fy
    cy[ar, m + 3] = fy
    cx[ar, k + 2] = 1.0 - fx
    cx[ar, k + 3] = fx

    z9 = np.concatenate([params, np.ones((N, 1), np.float32)], axis=1) * gbool
    z_exp = (cy[:, :, None, None] * cx[:, None, :, None] * z9[:, None, None, :])
    z_sum = z_exp.reshape(P_TOTAL, M, K).sum(axis=1)       # (P, 324) a-major

    # per-ptile chunk-set over live sources (chunk of a-value v is v>>1)
    live = (gbool.reshape(P_TOTAL, M) != 0)
    mp = m.reshape(P_TOTAL, M)
    csets = []
    for p in range(P_TOTAL):
        s = set()
        for q in range(M):
            if live[p, q]:
                mm = mp[p, q] + 2
                s.add(int(mm) >> 1)
                s.add((int(mm) + 1) >> 1)
        csets.append(frozenset(s))
    rank = np.array([_CLASS_RANK[cs] for cs in csets])
    perm = np.argsort(rank, kind="stable")                 # sorted global rank
    # round-robin deal: rank r -> core r % 8, column r // 8
    perm_rr = perm.reshape(PT, N_CORES).T                  # (core, col)

    sched = []
    for bi in range(NB):
        b0 = bi * 128
        bs = min(128, PT - b0)
        S = set()
        for r in perm[b0 * N_CORES:(b0 + bs) * N_CORES]:
            S |= csets[r]
        sched.append(tuple(sorted(int(c) for c in S)))
    sched = tuple(sched)

    zbatches = {c: [bi for bi in range(NB) if c in sched[bi]]
                for c in range(NCH)}
    ZW = max(1, sum(len(v) for v in zbatches.values())) * 128
    z_blk = np.zeros((N_CORES, CHUNK, ZW), np_dtype)
    zs = z_sum.astype(np_dtype)
    zoff = {c: sum(len(zbatches[cc]) for cc in range(c)) * 128
            for c in range(NCH)}
    for cr in range(N_CORES):
        zp = zs[perm_rr[cr]]                               # (PT, 324)
        for c in range(NCH):
            for i, bi in enumerate(zbatches[c]):
                b0 = bi * 128
                bs = min(128, PT - b0)
                z_blk[cr, :, zoff[c] + i * 128:zoff[c] + i * 128 + bs] = \
                    zp[b0:b0 + bs, c * CHUNK:(c + 1) * CHUNK].T

    canvas9 = np.zeros((F, CANVAS, CANVAS), np.float32)
    canvas9[:8, 3:54, 3:54] = W.reshape(8, 51, 51)
    canvas9[8, 3:54, 3:54] = b.reshape(51, 51)
    sw = np.lib.stride_tricks.sliding_window_view(
        canvas9, (OUT_HW, OUT_HW), axis=(1, 2))
    Wfull = sw.transpose(1, 2, 0, 3, 4).reshape(K, COLS)
    Wexp = np.empty((CHUNK, NCH * COLS), np_dtype)
    for ci in range(NCH):
        Wexp[:, ci * COLS:(ci + 1) * COLS] = \
            Wfull[ci * CHUNK:(ci + 1) * CHUNK]

    dead = np.array([len(cs) == 0 for cs in csets])
    return z_blk, Wexp, sched, perm_rr, dead


def kernel(locs, galaxy_params, galaxy_bool, W_dec, b_dec, _trace=False):
    import ml_dtypes
    from concourse.bass_utils import run_bass_kernel_spmd

    np_dtype = {"bf16": ml_dtypes.bfloat16, "f32": np.float32}[_DT_NAME]

    z_blk, Wexp, sched, perm_rr, dead = _host_expand(
        locs, galaxy_params, galaxy_bool, W_dec, b_dec, np_dtype)

    nc = _get_program(_DT_NAME, sched)
    in_maps = [{"zt": z_blk[c], "wx": Wexp} for c in range(N_CORES)]
    kwargs = {}
    if _trace:
        kwargs["trace"] = True
    res = run_bass_kernel_spmd(nc, in_maps, core_ids=list(range(N_CORES)),
                               **kwargs)

    out = np.empty((P_TOTAL, COLS), np.float32)
    for c in range(N_CORES):
        out[perm_rr[c]] = np.asarray(
            res.results[c]["out"][:PT], dtype=np.float32)
    out[dead] = 0.0
    out = out.reshape(P_TOTAL, 1, OUT_HW, OUT_HW)
    if _trace:
        kernel._last_result = res
    return out, out
